# revision 1
# baseline (speedup 1.0000x reference)
"""DiT dual-stream attention (B=4, S=2048, D=1024, H=16, DK=DV=64) on 8 TRN2 cores.

Sharding: core i handles batch b = i//2 and head-group g = i%2 (8 heads each).
v2: phases 2+3 merged with qp-outer loop; projection and the pairwise
AllReduce run per 512-token slab, overlapped with the next slab's attention.
All matmuls in float32r (full PE rate, ~1.5e-4 rel err measured on HW).
"""

import os
import sys

for _p in ("/opt/trn_rl_repo", "/root/.axon_site/_ro/trn_rl_repo"):
    if os.path.isdir(_p) and _p not in sys.path:
        sys.path.insert(0, _p)

import numpy as np

import concourse.bass as bass
import concourse.tile as tile
from concourse import bacc, mybir


F32 = mybir.dt.float32
F32R = mybir.dt.float32r

N_CORES = 8
B, S, D = 4, 2048, 1024
H, DK, DV = 16, 64, 64
HL = 8          # local heads per core
FQK = HL * DK   # 512: local q/k width per stream (x or c)
NJ = D // 128   # 8 contraction d-tiles
NP = 4          # token panels of 512
PAN = S // NP   # 512
NT = S // 128   # 16 token tiles
SCALE = 1.0 / np.sqrt(np.float32(DK))


def _build_nc(reps=1):
    nc = bacc.Bacc("TRN2", num_devices=N_CORES)

    xt_in = nc.dram_tensor("xt", [D, S], F32R, kind="ExternalInput")
    ct_in = nc.dram_tensor("ct", [D, S], F32R, kind="ExternalInput")
    w_names = ["wqx", "wqc", "wkx", "wkc", "wvx", "wvc"]
    w_in = {n: nc.dram_tensor(n, [D, FQK], F32R, kind="ExternalInput") for n in w_names}
    wp_in = nc.dram_tensor("wp", [HL * 2 * DV, D], F32R, kind="ExternalInput")
    bias_in = nc.dram_tensor("bias", [D], F32, kind="ExternalInput")
    y_out = nc.dram_tensor("y", [S, D], F32, kind="ExternalOutput")

    # DRAM scratch
    qT_d = nc.dram_tensor("qT_d", [HL, 128, S], F32R)
    kT_d = nc.dram_tensor("kT_d", [HL, 128, S], F32R)
    v_d = nc.dram_tensor("v_d", [NT, 128, HL, 128], F32R)  # [ktile, tok, h, vd]
    # per-slab partial/reduced tensors (separate tensors -> no false deps)
    y_part = [nc.dram_tensor(f"y_part{i}", [PAN, D], F32) for i in range(NP)]
    y_red = [nc.dram_tensor(f"y_red{i}", [PAN, D], F32) for i in range(NP)]

    import contextlib

    with tile.TileContext(nc) as tc:
        for _rep in range(reps):
            with tc.tile_pool(name="consts", bufs=1) as consts:
                ones_f = consts.tile([128, 128], F32)
                nc.vector.memset(ones_f, 1.0)
                ones = consts.tile([128, 128], F32R)
                nc.vector.tensor_copy(ones, ones_f)

                # ---------------- phase 1: QKV projections ----------------
                with (
                    tc.tile_pool(name="wpool", bufs=1) as wpool,
                    tc.tile_pool(name="xtp", bufs=2) as xtp,
                    tc.tile_pool(name="stage1", bufs=4) as stage1,
                    tc.tile_pool(name="ps_mm", bufs=4, space="PSUM") as ps_mm,
                ):
                    w_sb = {}
                    # first q/k weights + panel 0 activations lead the DMA queue
                    for n in ("wqx", "wqc"):
                        w_sb[n] = wpool.tile(
                            [128, NJ, FQK], F32R, tag=f"w_{n}", name=f"w_{n}"
                        )
                        nc.sync.dma_start(
                            out=w_sb[n],
                            in_=w_in[n].rearrange("(j p) f -> p j f", p=128),
                        )
                    panels = {}
                    for p in range(NP):
                        xT = xtp.tile([128, NJ, PAN], F32R, tag="xT", name=f"xT{p}")
                        cT = xtp.tile([128, NJ, PAN], F32R, tag="cT", name=f"cT{p}")
                        panels[p] = (xT, cT)
                        if p == 0:
                            for src, dst in ((xt_in, xT), (ct_in, cT)):
                                nc.sync.dma_start(
                                    out=dst,
                                    in_=src.rearrange("(j p) t -> p j t", p=128)[
                                        :, :, 0:PAN
                                    ],
                                )
                    for n in ("wkx", "wkc", "wvx", "wvc"):
                        w_sb[n] = wpool.tile(
                            [128, NJ, FQK], F32R, tag=f"w_{n}", name=f"w_{n}"
                        )
                        nc.sync.dma_start(
                            out=w_sb[n],
                            in_=w_in[n].rearrange("(j p) f -> p j f", p=128),
                        )

                    for p in range(NP):
                        xT, cT = panels[p]
                        if p > 0:
                            for src, dst in ((xt_in, xT), (ct_in, cT)):
                                nc.sync.dma_start(
                                    out=dst,
                                    in_=src.rearrange("(j p) t -> p j t", p=128)[
                                        :, :, p * PAN : (p + 1) * PAN
                                    ],
                                )

                        # q/k -> [f, tok] layout, scattered into qT_d/kT_d
                        for wn, src, dst_d, poff in (
                            ("wqx", xT, qT_d, 0),
                            ("wqc", cT, qT_d, 64),
                            ("wkx", xT, kT_d, 0),
                            ("wkc", cT, kT_d, 64),
                        ):
                            for fi in range(4):
                                ps = ps_mm.tile([128, PAN], F32, tag="ps1", name="ps")
                                for j in range(NJ):
                                    nc.tensor.matmul(
                                        ps,
                                        w_sb[wn][:, j, fi * 128 : (fi + 1) * 128],
                                        src[:, j, :],
                                        start=(j == 0),
                                        stop=(j == NJ - 1),
                                    )
                                st = stage1.tile([128, PAN], F32R, tag="qkstage")
                                nc.vector.tensor_copy(st, ps)
                                nc.sync.dma_start(
                                    out=dst_d[
                                        2 * fi, poff : poff + 64, p * PAN : (p + 1) * PAN
                                    ],
                                    in_=st[0:64, :],
                                )
                                nc.sync.dma_start(
                                    out=dst_d[
                                        2 * fi + 1,
                                        poff : poff + 64,
                                        p * PAN : (p + 1) * PAN,
                                    ],
                                    in_=st[64:128, :],
                                )

                        # v -> [tok, f] layout
                        for wn, src, voff in (("wvx", xT, 0), ("wvc", cT, 64)):
                            for tt in range(4):
                                ps = ps_mm.tile([128, FQK], F32, tag="ps1", name="ps")
                                for j in range(NJ):
                                    nc.tensor.matmul(
                                        ps,
                                        src[:, j, tt * 128 : (tt + 1) * 128],
                                        w_sb[wn][:, j, :],
                                        start=(j == 0),
                                        stop=(j == NJ - 1),
                                    )
                                st = stage1.tile([128, FQK], F32R, tag="vstage")
                                nc.vector.tensor_copy(st, ps)
                                nc.sync.dma_start(
                                    out=v_d[p * 4 + tt, :, :, voff : voff + 64],
                                    in_=st.rearrange("p (h d) -> p h d", h=HL),
                                )

                # -------- phases 2+3 merged: attention + proj, slab-pipelined --------
                with (
                    tc.tile_pool(name="kres", bufs=1) as kres,
                    tc.tile_pool(name="wp3", bufs=1) as wp3,
                    tc.tile_pool(name="vstr", bufs=2) as vstr,
                    tc.tile_pool(name="qstr", bufs=3) as qstr,
                    tc.tile_pool(name="aores", bufs=1) as aores,
                    tc.tile_pool(name="expp", bufs=6) as expp,
                    tc.tile_pool(name="small2", bufs=3) as small2,
                    tc.tile_pool(name="accp", bufs=2) as accp,
                    tc.tile_pool(name="y3", bufs=4) as y3,
                    tc.tile_pool(name="ps_s", bufs=4, space="PSUM") as ps_s,
                    tc.tile_pool(name="ps_sum", bufs=1, space="PSUM") as ps_sum,
                    tc.tile_pool(name="ps_out", bufs=2, space="PSUM") as ps_out,
                    tc.tile_pool(name="ps_y", bufs=1, space="PSUM") as ps_y,
                ):
                    # resident: all-head K (64KB/part), W_proj (32KB/part), bias
                    k_sb = kres.tile([128, HL, S], F32R, tag="k_sb")
                    for h in range(HL):
                        nc.sync.dma_start(out=k_sb[:, h, :], in_=kT_d[h])

                    wp_sb = wp3.tile([128, HL, D], F32R, tag="wp_sb")
                    nc.sync.dma_start(
                        out=wp_sb, in_=wp_in.rearrange("(j p) f -> p j f", p=128)
                    )

                    bias_b = wp3.tile([128, D], F32, tag="bias_b")
                    b_ap = bias_in[:]
                    nc.sync.dma_start(
                        out=bias_b,
                        in_=bass.AP(
                            tensor=b_ap.tensor,
                            offset=b_ap.offset,
                            ap=[[0, 128]] + [list(p) for p in b_ap.ap],
                        ),
                    )

                    for qp in range(NP):
                        ao_sb = aores.tile([128, HL, PAN], F32R, tag="ao_sb")
                        for h in range(HL):
                            qh = qstr.tile([128, PAN], F32R, tag="qh")
                            nc.sync.dma_start(
                                out=qh, in_=qT_d[h, :, qp * PAN : (qp + 1) * PAN]
                            )
                            vh = vstr.tile([128, NT, 128], F32R, tag="vh")
                            nc.sync.dma_start(
                                out=vh, in_=v_d[:, :, h, :].rearrange("k p d -> p k d")
                            )
                            p_sum = ps_sum.tile([128, PAN], F32, tag="p_sum")
                            p_out = ps_out.tile([128, PAN], F32, tag="p_out")
                            # exp tiles are summed over k-tiles on DVE (f32);
                            # one ones-matmul on the accumulated tile then
                            # partition-sums the final [128, PAN] (saves 15/16
                            # of the denominator matmul work on PE)
                            acc = accp.tile([128, PAN], F32, tag="acc")
                            acc_r = accp.tile([128, PAN], F32R, tag="acc_r")
                            ex0 = None
                            for kt in range(NT):
                                p_s = ps_s.tile([128, PAN], F32, tag="p_s")
                                nc.tensor.matmul(
                                    p_s,
                                    k_sb[:, h, kt * 128 : (kt + 1) * 128],
                                    qh,
                                    start=True,
                                    stop=True,
                                )
                                ex = expp.tile([128, PAN], F32R, tag="ex")
                                nc.scalar.activation(
                                    out=ex,
                                    in_=p_s,
                                    func=mybir.ActivationFunctionType.Exp,
                                    scale=float(SCALE),
                                )
                                if kt == 0:
                                    ex0 = ex
                                elif kt == 1:
                                    nc.vector.tensor_add(
                                        acc, ex0.bitcast(F32), ex.bitcast(F32)
                                    )
                                elif kt < NT - 1:
                                    nc.vector.tensor_add(acc, acc, ex.bitcast(F32))
                                else:
                                    nc.vector.tensor_add(acc_r, acc, ex.bitcast(F32))
                                nc.tensor.matmul(
                                    p_out,
                                    vh[:, kt, :],
                                    ex,
                                    start=(kt == 0),
                                    stop=(kt == NT - 1),
                                )
                            nc.tensor.matmul(p_sum, ones, acc_r, start=True, stop=True)
                            inv = small2.tile([128, PAN], F32, tag="inv")
                            nc.vector.reciprocal(inv, p_sum)
                            nc.vector.tensor_mul(ao_sb[:, h, :], p_out, inv)

                        # projection for this 512-token slab
                        for tt4 in range(4):
                            for do in range(2):
                                ps = ps_y.tile([128, 512], F32, tag="ps_y")
                                for fi in range(HL):
                                    nc.tensor.matmul(
                                        ps,
                                        ao_sb[:, fi, tt4 * 128 : (tt4 + 1) * 128],
                                        wp_sb[:, fi, do * 512 : (do + 1) * 512],
                                        start=(fi == 0),
                                        stop=(fi == HL - 1),
                                    )
                                yt = y3.tile([128, 512], F32, tag="yt")
                                nc.vector.tensor_add(
                                    yt, ps, bias_b[:, do * 512 : (do + 1) * 512]
                                )
                                nc.sync.dma_start(
                                    out=y_part[qp][
                                        tt4 * 128 : (tt4 + 1) * 128,
                                        do * 512 : (do + 1) * 512,
                                    ],
                                    in_=yt,
                                )

                        # pairwise all-reduce of this slab, overlapped with next slab
                        nc.gpsimd.collective_compute(
                            "AllReduce",
                            mybir.AluOpType.add,
                            replica_groups=[[0, 1], [2, 3], [4, 5], [6, 7]],
                            ins=[y_part[qp][:]],
                            outs=[y_red[qp][:]],
                        )
                        nc.gpsimd.dma_start(
                            out=y_out[qp * PAN : (qp + 1) * PAN, :], in_=y_red[qp][:]
                        )

    nc.finalize()
    return nc


_NC = {}


def _get_nc(reps=1):
    global _NC
    if _NC is None:
        _NC = {}
    if reps not in _NC:
        _NC[reps] = _build_nc(reps)
    return _NC[reps]


def _shard_inputs(inputs):
    x = np.ascontiguousarray(inputs["x"], dtype=np.float32)
    c = np.ascontiguousarray(inputs["c"], dtype=np.float32)
    wq_x, wk_x, wv_x = inputs["Wq_x"], inputs["Wk_x"], inputs["Wv_x"]
    wq_c, wk_c, wv_c = inputs["Wq_c"], inputs["Wk_c"], inputs["Wv_c"]
    w_proj, b_proj = inputs["W_proj"], inputs["b_proj"]

    in_maps = []
    for core in range(N_CORES):
        b, g = core // 2, core % 2
        fs = slice(g * FQK, (g + 1) * FQK)
        m = {
            "xt": np.ascontiguousarray(x[b].T),
            "ct": np.ascontiguousarray(c[b].T),
            "wqx": np.ascontiguousarray(wq_x[:, fs], dtype=np.float32),
            "wqc": np.ascontiguousarray(wq_c[:, fs], dtype=np.float32),
            "wkx": np.ascontiguousarray(wk_x[:, fs], dtype=np.float32),
            "wkc": np.ascontiguousarray(wk_c[:, fs], dtype=np.float32),
            "wvx": np.ascontiguousarray(wv_x[:, fs], dtype=np.float32),
            "wvc": np.ascontiguousarray(wv_c[:, fs], dtype=np.float32),
            "wp": np.ascontiguousarray(
                w_proj[g * HL * 2 * DV : (g + 1) * HL * 2 * DV, :], dtype=np.float32
            ),
            "bias": (
                np.ascontiguousarray(b_proj, dtype=np.float32)
                if g == 0
                else np.zeros((D,), np.float32)
            ),
        }
        in_maps.append(m)
    return in_maps


def kernel(**inputs) -> np.ndarray:
    from concourse.bass_utils import run_bass_kernel_spmd

    nc = _get_nc()
    in_maps = _shard_inputs(inputs)
    res = run_bass_kernel_spmd(nc, in_maps, list(range(N_CORES)))
    y = np.stack([res.results[2 * b]["y"] for b in range(B)], axis=0)
    return y.astype(np.float32)

